# revision 24
# baseline (speedup 1.0000x reference)
"""Trainium2 Bass kernel for nn_ConformalLayers (8-core data-parallel).

Math (reference):
    X = x.reshape(B, 3072).T                         # [3072, B]
    Y = M @ X                                        # [16384, B]
    Y_extra = s * ||X||_col + sum((T @ X) * X, 0)    # [1, B]
    out = (Y / Y_extra).T.reshape(B, 64, 16, 16)

Sharding: batch B=4096 split as 512 columns per core; M / T / s replicated.

fp8 DoubleRow strategy: on TRN2 the PE runs float8e4 matmuls in DoubleRow
perf mode at 0.5 cycles per output row while contracting TWO 128-deep
k-subtiles per instruction (lhsT [128,2,M], rhs [128,2,N]) — 4x the
fp16 GEMM throughput per the hardware cost model. Raw e4m3 quantization
(sigma ~2.7e-2 per operand) would exceed the 2e-2 error budget, so the
main GEMM uses a 3-term error-compensated split:

    M@X ~= M_hi@X_hi + M_hi@X_lo + M_lo@X_hi       (residual ~1.2e-3)

with M_hi = fp8(M), M_lo = fp8(M - M_hi), same for X. This costs 0.75x
the fp16 PE cycles. The T quadratic form q = x^T T x is attenuated by
the normalizer (|q| << s*||x||), so a single uncompensated fp8 pass
suffices there (output contribution ~1e-3). M, T and s are pre-scaled
by 2^8 host-side (keeps fp8 operands out of the subnormal range); the
scale cancels exactly: psum = 256*M@X, ye = 256*Y_extra.

Per-core PE work: GEMM1 3*12*32*4 + TT 12*6*4 DoubleRow matmuls of 256
cycles = 1,253,376 cycles ~= 522 us @ 2.4 GHz (vs 1,867,776 ~= 811 us
for the fp16 baseline). DMA ~400 MB/s-serialized ~= 415 us, overlapped.
DMA issue is split across SP (xhi, tt, M_hi), ACT (xn, xlo, M_lo) and
DVE (out) sequencers so no single queue serializes the streams.
"""

import os
from contextlib import ExitStack

import numpy as np

import concourse.bass as bass
import concourse.tile as tile
from concourse import bacc, mybir
from concourse import bass_utils

B = 4096
IN_NUMEL = 3072
OUT_NUMEL = 16384
OUT_DIMS = (64, 16, 16)
N_CORES = 8
BC = B // N_CORES            # 512 batch columns per core
P = 128
NB = BC // P                 # 4 batch blocks of 128
KT = IN_NUMEL // P           # 24 k-subtiles
KP = KT // 2                 # 12 k-pairs (DoubleRow does 2 subtiles/matmul)
NT_OUT = OUT_NUMEL // 512    # 32 n-tiles over M rows
NT_IN = IN_NUMEL // 512      # 6 n-tiles over T columns
SCALE = 256.0                # power-of-2 prescale on M, T, s (cancels exactly)

F8 = mybir.dt.float8e4
_F8_NP = mybir.dt.np(F8)

_PROGRAM = None
# Dev bisect knob: full | nolo (single-pass fp8) | noTT (rt = 1)
_VARIANT = os.environ.get("KERNEL_VARIANT", "full")


def _build_program():
    nc = bacc.Bacc(
        "TRN2",
        target_bir_lowering=False,
        debug=False,
        enable_asserts=False,
        num_devices=N_CORES,
        enable_partition_id=False,
    )
    xhi = nc.dram_tensor("xhi", (IN_NUMEL, BC), F8, kind="ExternalInput")
    xlo = nc.dram_tensor("xlo", (IN_NUMEL, BC), F8, kind="ExternalInput")
    xn = nc.dram_tensor("xn", (BC, IN_NUMEL), mybir.dt.float16, kind="ExternalInput")
    mthi = nc.dram_tensor("mthi", (IN_NUMEL, OUT_NUMEL), F8, kind="ExternalInput")
    mtlo = nc.dram_tensor("mtlo", (IN_NUMEL, OUT_NUMEL), F8, kind="ExternalInput")
    NWBLK = (IN_NUMEL // 512) * (IN_NUMEL // 512 + 1) // 2   # 21 upper-tri blocks
    wpk = nc.dram_tensor("wpk", (NWBLK * 512, 512), F8, kind="ExternalInput")
    sc = nc.dram_tensor("sc", (P, 1), mybir.dt.float32, kind="ExternalInput")
    out = nc.dram_tensor("out", (BC, OUT_NUMEL), mybir.dt.float32, kind="ExternalOutput")

    f32 = mybir.dt.float32
    Alu = mybir.AluOpType
    Act = mybir.ActivationFunctionType
    DR = mybir.MatmulPerfMode.DoubleRow

    MT_BUFS = int(os.environ.get("KERNEL_MT_BUFS", "8"))
    TT_BUFS = int(os.environ.get("KERNEL_TT_BUFS", "6"))

    with tile.TileContext(nc) as tc:
        with ExitStack() as ctx:
            small = ctx.enter_context(tc.tile_pool(name="small", bufs=1))
            xpool = ctx.enter_context(tc.tile_pool(name="xp", bufs=1))
            scratch = ctx.enter_context(tc.tile_pool(name="scr", bufs=2))
            mt_pool = ctx.enter_context(tc.tile_pool(name="mtp", bufs=MT_BUFS))
            tt_pool = ctx.enter_context(tc.tile_pool(name="ttp", bufs=TT_BUFS))
            out_pool = ctx.enter_context(tc.tile_pool(name="outp", bufs=3))
            psum = ctx.enter_context(tc.tile_pool(name="psum", bufs=2, space="PSUM"))

            # --- static operands -------------------------------------------
            # xhi chunks interleave with the t=0 tt tiles (pipelined front)
            xhi_sb = xpool.tile([P, KT, BC], F8)
            xhi_ap = xhi.ap().rearrange("(t p) b -> p t b", p=P)
            s_sb = small.tile([P, 1], f32)

            # xn + xlo on the ACT queue so they don't delay SP's tt stream
            xn_t = xpool.tile([P, NB, IN_NUMEL], mybir.dt.float16, name="xn_t")
            xn_ap = xn.ap().rearrange("(t p) k -> p t k", p=P)

            np2 = small.tile([P, NB * NT_IN], f32)   # per-chunk sum(x^2)
            qp = small.tile([P, NB * NT_IN], f32)    # per-chunk sum(Z*x)
            n2 = small.tile([P, NB], f32)
            qv = small.tile([P, NB], f32)
            sn = small.tile([P, NB], f32)
            ye = small.tile([P, NB], f32)
            rt = small.tile([P, NB], f32)            # 1 / (256 * Y_extra)

            xlo_sb = xpool.tile([P, KT, BC], F8)
            xlo_ap = xlo.ap().rearrange("(t p) b -> p t b", p=P)

            def load_xn_chunk(c):
                # xn chunk + its norm partials, issued on ACT just in time
                nc.scalar.dma_start(
                    xn_t[:, :, c * 512 : (c + 1) * 512],
                    xn_ap[:, :, c * 512 : (c + 1) * 512],
                )
                for b in range(NB):
                    scr = scratch.tile([P, 512], f32, tag="sq")
                    nc.scalar.activation(
                        scr[:],
                        xn_t[:, b, c * 512 : (c + 1) * 512],
                        Act.Square,
                        accum_out=np2[:, b * NT_IN + c : b * NT_IN + c + 1],
                    )

            def lhs_hi(j, m):
                return xhi_sb[:, 2 * j : 2 * j + 2, m * P : (m + 1) * P]

            def lhs_lo(j, m):
                return xlo_sb[:, 2 * j : 2 * j + 2, m * P : (m + 1) * P]

            # --- TT phase: q[b] = x^T S x, S = (T+T^T)/2 -------------------
            # Only upper-triangle 512-blocks of S are streamed (off-diagonal
            # blocks pre-doubled host-side): 21 block-GEMMs instead of 36.
            KD = 4                        # k-pairs per GEMM1 DMA tile
            NJT = KP // KD                # 3 deep tiles over K
            wpk_ap = wpk.ap()
            if _VARIANT != "noTT":
                for u in range(NT_IN):
                    load_xn_chunk(u)
                    if u == 0:
                        nc.sync.dma_start(
                            xhi_sb[:, 0:8, :], xhi_ap[:, 0:8, :]
                        )
                    elif u in (1, 3):      # xhi chunks 1, 2 as k-range grows
                        g = (u + 1) // 2
                        nc.sync.dma_start(
                            xhi_sb[:, g * 8 : (g + 1) * 8, :],
                            xhi_ap[:, g * 8 : (g + 1) * 8, :],
                        )
                    if u >= NT_IN - NJT:   # xlo chunks trail in during TT
                        g = u - (NT_IN - NJT)
                        nc.scalar.dma_start(
                            xlo_sb[:, g * 8 : (g + 1) * 8, :],
                            xlo_ap[:, g * 8 : (g + 1) * 8, :],
                        )
                    ps = [psum.tile([P, 512], f32, tag=f"ps{m}", name=f"ps{m}")
                          for m in range(NB)]
                    row0 = (u * (u + 1) // 2) * 512
                    for t in range(u + 1):
                        wt = tt_pool.tile([P, 4, 512], F8, tag="tt", name="wt")
                        nc.sync.dma_start(
                            wt[:],
                            wpk_ap[row0 + t * 512 : row0 + (t + 1) * 512, :]
                            .rearrange("(q p) n -> p q n", p=P),
                        )
                        for jp in range(2):
                            j = 2 * t + jp
                            for m in range(NB):
                                nc.tensor.matmul(
                                    ps[m][:], lhs_hi(j, m),
                                    wt[:, 2 * jp : 2 * jp + 2, :],
                                    start=(t == 0 and jp == 0),
                                    stop=(t == u and jp == 1), perf_mode=DR,
                                )
                    for m in range(NB):
                        scr = scratch.tile([P, 512], f32, tag="ttred")
                        nc.vector.tensor_mul(
                            scr[:], ps[m][:], xn_t[:, m, u * 512 : (u + 1) * 512]
                        )
                        nc.vector.tensor_reduce(
                            qp[:, m * NT_IN + u : m * NT_IN + u + 1], scr[:],
                            mybir.AxisListType.X, Alu.add,
                        )
                nc.sync.dma_start(s_sb[:], sc.ap())
                for b in range(NB):
                    nc.vector.tensor_reduce(
                        n2[:, b : b + 1], np2[:, b * NT_IN : (b + 1) * NT_IN],
                        mybir.AxisListType.X, Alu.add,
                    )
                nc.scalar.sqrt(sn[:], n2[:])
                for b in range(NB):
                    nc.vector.tensor_reduce(
                        qv[:, b : b + 1], qp[:, b * NT_IN : (b + 1) * NT_IN],
                        mybir.AxisListType.X, Alu.add,
                    )
                # ye = sn * (256*s) + qv  == 256 * Y_extra
                nc.vector.scalar_tensor_tensor(
                    out=ye[:], in0=sn[:], scalar=s_sb[:, 0:1], in1=qv[:],
                    op0=Alu.mult, op1=Alu.add,
                )
                nc.vector.reciprocal(rt[:], ye[:])
            else:
                nc.sync.dma_start(xhi_sb[:], xhi_ap)
                nc.sync.dma_start(s_sb[:], sc.ap())
                for c in range(NT_IN):
                    load_xn_chunk(c)
                nc.scalar.dma_start(xlo_sb[:], xlo_ap)
                nc.vector.memset(rt[:], 1.0 / SCALE)
            # --- GEMM1: out = (M@X) / Y_extra ------------------------------
            mthi_ap = mthi.ap().rearrange("(a q p) n -> p a q n", p=P, q=2 * KD)
            mtlo_ap = mtlo.ap().rearrange("(a q p) n -> p a q n", p=P, q=2 * KD)
            out_ap = out.ap().rearrange("(m p) n -> p m n", p=P)
            for t in range(NT_OUT):
                cs = slice(t * 512, (t + 1) * 512)
                ps = [psum.tile([P, 512], f32, tag=f"ps{m}", name=f"ps{m}")
                      for m in range(NB)]
                for jt in range(NJT):
                    th = mt_pool.tile([P, 2 * KD, 512], F8, tag="mh", name="th")
                    nc.sync.dma_start(th[:], mthi_ap[:, jt, :, cs])
                    if _VARIANT != "nolo":
                        tl = mt_pool.tile([P, 2 * KD, 512], F8, tag="ml", name="tl")
                        nc.scalar.dma_start(tl[:], mtlo_ap[:, jt, :, cs])
                    for jp in range(KD):
                        j = jt * KD + jp
                        qs = slice(2 * jp, 2 * jp + 2)
                        last = (j == KP - 1)
                        for m in range(NB):
                            nc.tensor.matmul(
                                ps[m][:], lhs_hi(j, m), th[:, qs, :],
                                start=(j == 0), stop=(last and _VARIANT == "nolo"),
                                perf_mode=DR,
                            )
                        if _VARIANT != "nolo":
                            for m in range(NB):
                                nc.tensor.matmul(
                                    ps[m][:], lhs_lo(j, m), th[:, qs, :],
                                    start=False, stop=False, perf_mode=DR,
                                )
                            for m in range(NB):
                                nc.tensor.matmul(
                                    ps[m][:], lhs_hi(j, m), tl[:, qs, :],
                                    start=False, stop=last, perf_mode=DR,
                                )
                ot = out_pool.tile([P, NB, 512], f32, tag="ot", name="ot")
                for m in range(NB):
                    nc.vector.tensor_scalar_mul(
                        ot[:, m, :], ps[m][:], rt[:, m : m + 1]
                    )
                    nc.sync.dma_start(out_ap[:, m : m + 1, cs], ot[:, m : m + 1, :])

    nc.compile()
    return nc


def get_program():
    global _PROGRAM
    if _PROGRAM is None:
        _PROGRAM = _build_program()
    return _PROGRAM


def make_in_maps(x, cached_matrix, cached_matrix_extra, cached_tensor_extra):
    xf = np.ascontiguousarray(np.asarray(x, dtype=np.float32).reshape(B, IN_NUMEL))
    MT = np.ascontiguousarray(np.asarray(cached_matrix, dtype=np.float32).T) * np.float32(SCALE)
    MThi = MT.astype(_F8_NP)
    MTlo = (MT - MThi.astype(np.float32)).astype(_F8_NP)
    del MT
    Tm = np.asarray(cached_tensor_extra, dtype=np.float32)
    S = (Tm + Tm.T) * np.float32(0.5 * SCALE)
    del Tm
    NT6 = IN_NUMEL // 512
    wblocks = []
    for u in range(NT6):
        for t in range(u + 1):
            blk = S[t * 512 : (t + 1) * 512, u * 512 : (u + 1) * 512]
            wblocks.append((blk * 2 if t < u else blk).astype(_F8_NP))
    WPK = np.ascontiguousarray(np.concatenate(wblocks, axis=0))
    del S, wblocks
    s = np.full((P, 1),
                np.float32(np.asarray(cached_matrix_extra).reshape(-1)[0]) * np.float32(SCALE),
                dtype=np.float32)
    in_maps = []
    for c in range(N_CORES):
        sl = slice(c * BC, (c + 1) * BC)
        XT = np.ascontiguousarray(xf[sl, :].T)           # [3072, 512] f32
        Xhi = XT.astype(_F8_NP)
        Xlo = (XT - Xhi.astype(np.float32)).astype(_F8_NP)
        in_maps.append({
            "xhi": Xhi,
            "xlo": Xlo,
            "xn": xf[sl, :].astype(np.float16),
            "mthi": MThi,
            "mtlo": MTlo,
            "wpk": WPK,
            "sc": s,
        })
    return in_maps


_AXON_EXEC = None


def _build_axon_exec():
    """Staged PJRT runner for the axon path.

    run_bass_kernel_spmd's axon redirect concatenates all per-core inputs into
    single giant host arrays (1.6 GB for the replicated cached_matrix), which
    hits a pathologically slow transfer path in the relay. Instead we stage
    shards/replicas with individually-sized device_puts and run the same
    bass_exec custom call through shard_map ourselves.
    """
    import jax
    from jax.sharding import Mesh, NamedSharding, PartitionSpec
    from jax.experimental.shard_map import shard_map
    from concourse import bass2jax

    nc = get_program()
    bass2jax.install_neuronx_cc_hook()

    in_names, out_names, out_avals = [], [], []
    for alloc in nc.m.functions[0].allocations:
        if not isinstance(alloc, mybir.MemoryLocationSet):
            continue
        name = alloc.memorylocations[0].name
        if alloc.kind == "ExternalInput":
            in_names.append(name)
        elif alloc.kind == "ExternalOutput":
            out_names.append(name)
            out_avals.append(
                jax.core.ShapedArray(
                    tuple(alloc.tensor_shape), mybir.dt.np(alloc.dtype)
                )
            )
    all_in_names = in_names + out_names
    # per-input sharding: batch-sharded vs replicated model caches
    sharded_inputs = {"xhi", "xlo", "xn"}

    def _body(*args):
        outs = bass2jax._bass_exec_p.bind(
            *args,
            out_avals=tuple(out_avals),
            in_names=tuple(all_in_names),
            out_names=tuple(out_names),
            lowering_input_output_aliases=(),
            sim_require_finite=True,
            sim_require_nnan=True,
            nc=nc,
        )
        return tuple(outs)

    devices = jax.devices()[:N_CORES]
    mesh = Mesh(np.asarray(devices), ("core",))
    core_spec = PartitionSpec("core")
    repl_spec = PartitionSpec()
    in_specs = tuple(
        core_spec if n in sharded_inputs else repl_spec for n in in_names
    ) + (core_spec,) * len(out_names)
    sharded = jax.jit(
        shard_map(
            _body,
            mesh=mesh,
            in_specs=in_specs,
            out_specs=(core_spec,) * len(out_names),
            check_rep=False,
        ),
        keep_unused=True,
    )

    def stage(in_maps):
        import concurrent.futures as cf

        core_sh = NamedSharding(mesh, core_spec)
        repl_sh = NamedSharding(mesh, repl_spec)

        def stage_one(name):
            if name in sharded_inputs:
                glob = np.concatenate([m[name] for m in in_maps], axis=0)
                return jax.device_put(glob, core_sh)
            return jax.device_put(in_maps[0][name], repl_sh)

        with cf.ThreadPoolExecutor(len(in_names)) as ex:
            staged = list(ex.map(stage_one, in_names))
        for s in staged:
            s.block_until_ready()
        zeros = [
            jax.jit(
                lambda a=a: jax.numpy.zeros((N_CORES * a.shape[0], *a.shape[1:]), a.dtype),
                out_shardings=core_sh,
            )()
            for a in out_avals
        ]
        return staged + zeros

    def execute(staged):
        outs = sharded(*staged)
        jax.block_until_ready(outs)
        return outs

    def run(in_maps):
        return execute(stage(in_maps))

    _state = {"sharded": sharded, "stage": stage, "execute": execute, "run": run}
    return _state


def get_axon_exec():
    global _AXON_EXEC
    if _AXON_EXEC is None:
        _AXON_EXEC = _build_axon_exec()
    return _AXON_EXEC


def kernel(x, cached_matrix, cached_matrix_extra, cached_tensor_extra):
    from concourse._compat import axon_active

    in_maps = make_in_maps(x, cached_matrix, cached_matrix_extra, cached_tensor_extra)
    if axon_active():
        outs = get_axon_exec()["run"](in_maps)
        out = np.asarray(outs[0])  # [B, OUT_NUMEL]
    else:
        nc = get_program()
        res = bass_utils.run_bass_kernel_spmd(nc, in_maps, core_ids=list(range(N_CORES)))
        out = np.concatenate([r["out"] for r in res.results], axis=0)
    return np.ascontiguousarray(out).reshape(B, *OUT_DIMS)
